# revision 26
# baseline (speedup 1.0000x reference)
"""Trainium2 Bass kernel for nn_DIDAModule (dense_cnn).

Math: the per-sample "dynamic" depthwise kernels are affine in the channel
gate g:  kern1 = g*A1 + B1  with  A1 = wk*wck, B1 = bk*wck + bck  (5x5) and
A2 = wk2*wck2, B2 = bk2*wck2 + bck2 (3x3, dilation 2).  Per-channel scaling
commutes with the (channel-shared) depthwise convs, so

    o1 = conv_A1(g*f) + conv_B1(f)      o2 = conv_A2(g*f) + conv_B2(f)
    y  = [W_fuse @ o1 + b_fuse ; W_fuse @ o2 + b_fuse]

The four static depthwise convs are banded spatial matmuls on the Tensor
engine: spatial-major layout, flat 128-pixel blocks, 7 phase classes
(128 mod 56 = 16, period 7), 3 band position matrices (prev/self/next
block) per phase per kernel, built host side from the conv geometry.

Layout changes between channel-major (conv1, fuse) and spatial-major
(banded sweeps) are done with PE-array transposes (is_transpose matmuls
against an identity), packed 8 tiles per PSUM bank — no DMA-xbar
transposes, no DRAM staging, no write-landing delay chains.  Everything
is bf16 with fp32 PSUM accumulation; x is cast to bf16 host-side and y
is returned as bf16 and cast back host-side.

Sharding: data-parallel over batch N across the 8 cores (4 samples each),
weights replicated.
"""

import numpy as np

# ---------------------------------------------------------------- dims
N, C, H, W = 32, 512, 56, 56
CM, K1, K2, P2 = 128, 5, 3, 256
HW = H * W            # 3136
SP = 3200             # padded spatial: 25 blocks of 128
NB = 25
PH = 7                # phase classes
NCORES = 8
NPC = N // NCORES     # samples per core
EW = 448              # conv1 / fuse strip width (7 per map)
NE = 7

_CACHE = {}


# ---------------------------------------------------------------- host prep
def _build_T(K2d, dil):
    """Banded conv matrices T[phase, pos, k_in, m_out] for flat 128-blocks."""
    kh = K2d.shape[0]
    r = (kh - 1) // 2 * dil
    T = np.zeros((PH, 3, 128, 128), np.float32)
    for p in range(PH):
        bref = 7 + p              # interior reference block of this phase
        for pos, d in enumerate((-1, 0, 1)):
            for m in range(128):
                s_out = bref * 128 + m
                ro, wo = divmod(s_out, W)
                for k in range(128):
                    s_in = (bref + d) * 128 + k
                    ri, wi = divmod(s_in, W)
                    di, dj = ri - ro, wi - wo
                    if (abs(di) <= r and abs(dj) <= r
                            and di % dil == 0 and dj % dil == 0):
                        T[p, pos, k, m] = K2d[di // dil + (kh - 1) // 2,
                                              dj // dil + (kh - 1) // 2]
    return T


def _host_consts(inp):
    import ml_dtypes
    bf16 = ml_dtypes.bfloat16
    W_conv = np.asarray(inp["W_conv"], np.float32)     # [CM, C]
    W_fuse = np.asarray(inp["W_fuse"], np.float32)     # [P2, CM]
    A1 = (np.asarray(inp["wk"]) * float(inp["wck"])).reshape(K1, K1)
    B1 = (np.asarray(inp["bk"]) * float(inp["wck"]) + float(inp["bck"])).reshape(K1, K1)
    A2 = (np.asarray(inp["wk2"]) * float(inp["wck2"])).reshape(K2, K2)
    B2 = (np.asarray(inp["bk2"]) * float(inp["wck2"]) + float(inp["bck2"])).reshape(K2, K2)
    # T layout: [k_in(128part), kern(4), ph(7), pos(3), m_out(128)]
    T = np.stack([_build_T(A1.astype(np.float32), 1),
                  _build_T(B1.astype(np.float32), 1),
                  _build_T(A2.astype(np.float32), 2),
                  _build_T(B2.astype(np.float32), 2)])      # [4,7,3,128,128]
    T_h = np.ascontiguousarray(T.transpose(3, 0, 1, 2, 4)).reshape(128, 84 * 128)
    # conv1 lhsT chunks: [c_local(128part), kc(4), cm(128)]
    wconvT_h = np.ascontiguousarray(
        W_conv.T.reshape(4, 128, CM).transpose(1, 0, 2)).reshape(128, 4 * CM)
    # fuse lhsT chunks: [c(128part), chunk(2), o_local(128)]
    wfuseT_h = np.ascontiguousarray(
        W_fuse.T.reshape(CM, 2, 128)).reshape(CM, 256)
    # fuse bias per-partition columns: [o_local(128part), chunk(2)]
    bfuse_h = np.ascontiguousarray(
        np.asarray(inp["b_fuse"], np.float32).reshape(2, 128).T)
    return {
        "wconvT": wconvT_h.astype(bf16),
        "bconv": np.asarray(inp["b_conv"], np.float32).reshape(CM, 1),
        "Tmat": T_h.astype(bf16),
        "wfuseT": wfuseT_h.astype(bf16),
        "bfuse": bfuse_h,
        "ident": np.eye(128, dtype=bf16),
    }


# ---------------------------------------------------------------- bass module
def _build_module():
    from contextlib import ExitStack
    import concourse.bass as bass  # noqa: F401
    import concourse.mybir as mybir
    import concourse.tile as tile
    from concourse import bacc

    dt = mybir.dt
    AX = mybir.AxisListType
    AF = mybir.ActivationFunctionType

    nc = bacc.Bacc("TRN2", target_bir_lowering=False, debug=False)

    x_d = nc.dram_tensor("x", [NPC, C, HW], dt.bfloat16, kind="ExternalInput").ap()
    wconvT_d = nc.dram_tensor("wconvT", [128, 4 * CM], dt.bfloat16, kind="ExternalInput").ap()
    bconv_d = nc.dram_tensor("bconv", [CM, 1], dt.float32, kind="ExternalInput").ap()
    T_d = nc.dram_tensor("Tmat", [128, 84 * 128], dt.bfloat16, kind="ExternalInput").ap()
    wfuseT_d = nc.dram_tensor("wfuseT", [CM, 256], dt.bfloat16, kind="ExternalInput").ap()
    bfuse_d = nc.dram_tensor("bfuse", [128, 2], dt.float32, kind="ExternalInput").ap()
    ident_d = nc.dram_tensor("ident", [128, 128], dt.bfloat16, kind="ExternalInput").ap()
    # y layout: [n, m(2 branches), otile(2), o_local(128), s]
    y_d = nc.dram_tensor("y", [NPC, 2, 2, 128, HW], dt.bfloat16, kind="ExternalOutput").ap()

    with tile.TileContext(nc) as tc, ExitStack() as ctx:
        consts = ctx.enter_context(tc.tile_pool(name="consts", bufs=1))
        xpool = ctx.enter_context(tc.tile_pool(name="xp", bufs=11))
        fpool = ctx.enter_context(tc.tile_pool(name="fp", bufs=1))
        big = ctx.enter_context(tc.tile_pool(name="big", bufs=1))
        otp = ctx.enter_context(tc.tile_pool(name="otp", bufs=2))
        ystage = ctx.enter_context(tc.tile_pool(name="yst", bufs=4))
        small = ctx.enter_context(tc.tile_pool(name="small", bufs=4))
        psA = ctx.enter_context(tc.tile_pool(name="psA", bufs=4, space="PSUM"))
        psB = ctx.enter_context(tc.tile_pool(name="psB", bufs=4, space="PSUM"))

        def acc_tile():
            # uniform [128, 512] fp32 bank; callers slice/reshape views
            return psA.tile([128, 4, 128], dt.float32, tag="acc", name="acc",
                            bufs=3)

        def tp_tile():
            # uniform [128, 2*NPC, 128] bf16 bank for PE transposes
            return psB.tile([128, 2, NPC, 128], dt.bfloat16, tag="tp", name="tp",
                            bufs=3)

        def fu_tile():
            # dedicated fuse ring so fuse copies don't stall the sweep banks
            return psA.tile([128, 4, 128], dt.float32, tag="fu", name="fu",
                            bufs=2)

        # ---- constants to SBUF (small, immediately-needed ones first; the
        # heavy Tm and fuse consts are DMA'd after sample 0's x chunks)
        wconvT = consts.tile([128, 4, CM], dt.bfloat16)
        nc.sync.dma_start(out=wconvT, in_=wconvT_d)
        bconv = consts.tile([CM, 1], dt.float32)
        nc.sync.dma_start(out=bconv, in_=bconv_d)
        ident = consts.tile([128, 128], dt.bfloat16)
        nc.sync.dma_start(out=ident, in_=ident_d)
        Tm = consts.tile([128, 84, 128], dt.bfloat16)
        wfuseT = consts.tile([CM, 2, 128], dt.bfloat16)
        bfuse = consts.tile([128, 2], dt.float32)

        # ---- persistent spatial-major tensors
        # fT_all: [s_loc, map(0=f,1=f*g), b, j, c]
        fT_all = big.tile([128, 2, NB, NPC, 128], dt.bfloat16)
        # oc: [c, m(0=5x5,1=3x3d2), j, b, s_loc]
        oc = big.tile([128, 2, NPC, NB, 128], dt.bfloat16)

        # engine-alternating PSUM->SBUF copy (plain cast copy)
        _alt = [0]

        def copy_cast(dst, src):
            _alt[0] ^= 1
            if _alt[0]:
                nc.scalar.activation(dst, src, AF.Copy)
            else:
                nc.vector.tensor_copy(dst, src)

        def tslice(kid, ph, pos):
            return Tm[:, kid * 21 + ph * 3 + pos, :]

        # gate broadcast rows: gB_all[:, n, :] = g_n[c] replicated over partitions
        gB_all = big.tile([128, NPC, 128], dt.bfloat16)

        # ================= phase 1: conv1 + gate + forward transposes
        def load_x(n):
            xc = []
            for kc in range(4):
                xt = xpool.tile([128, HW], dt.bfloat16, tag="x")
                # stripe x across both HWDGE queues (SP + Activation); the
                # aggregate DMA rate is HBM-bound so depth, not queues, hides it
                eng = nc.sync if kc % 2 == 0 else nc.scalar
                eng.dma_start(out=xt, in_=x_d[n, kc * 128:(kc + 1) * 128, :])
                xc.append(xt)
            return xc

        xcs = {n: load_x(n) for n in range(3)}   # ~3 samples in the x ring
        for n in range(NPC):
            xc = xcs.pop(n)
            if n == 1:
                xcs[3] = load_x(3)
                # heavy consts: tail of the ACT queue, needed only at sweeps
                nc.scalar.dma_start(out=Tm, in_=T_d)
                nc.scalar.dma_start(out=wfuseT, in_=wfuseT_d)
                nc.scalar.dma_start(out=bfuse, in_=bfuse_d)
            # f tile: [c, s_padded]; pad columns must be zero
            f = fpool.tile([128, SP], dt.bfloat16, tag="f")
            nc.gpsimd.memset(f[:, HW:SP], 0.0)
            gpart = small.tile([128, NE], dt.float32, tag="gp")
            # sample 0: strip waves of 3 so the first matmuls only need x
            # chunk 0 (later chunks are still in flight); others strip-major
            waves = [(0, 3), (3, 6), (6, 7)] if n == 0 else [(e, e + 1)
                                                             for e in range(NE)]
            for w0, w1 in waves:
                pss = [acc_tile().rearrange("p a b -> p (a b)")[:, :EW]
                       for _ in range(w0, w1)]
                for kc in range(4):
                    for i, e in enumerate(range(w0, w1)):
                        nc.tensor.matmul(pss[i], wconvT[:, kc, :],
                                         xc[kc][:, e * EW:(e + 1) * EW],
                                         start=(kc == 0), stop=(kc == 3))
                for i, e in enumerate(range(w0, w1)):
                    nc.vector.reduce_sum(gpart[:, e:e + 1], pss[i], axis=AX.X)
                    nc.scalar.activation(f[:, e * EW:(e + 1) * EW], pss[i],
                                         AF.Relu, bias=bconv[:, 0:1], scale=1.0)
            gsum = small.tile([128, 1], dt.float32, tag="gs")
            nc.vector.reduce_sum(gsum, gpart, axis=AX.X)
            g = small.tile([128, 1], dt.bfloat16, tag="g")
            nc.scalar.activation(g, gsum, AF.Relu, bias=bconv[:, 0:1],
                                 scale=1.0 / HW)
            # forward transposes of f
            groups = [(b0, min(b0 + 8, NB)) for b0 in range(0, NB, 8)]
            for b0, b1 in groups:
                pt = tp_tile().rearrange("p a b c -> p (a b) c")
                for k, b in enumerate(range(b0, b1)):
                    nc.tensor.matmul(pt[:, k, :], f[:, b * 128:(b + 1) * 128],
                                     ident, is_transpose=True,
                                     start=(k == 0), stop=(b == b1 - 1))
                copy_cast(fT_all[:, 0, b0:b1, n, :], pt[:, 0:b1 - b0, :])
            # transpose g to a row, broadcast across partitions (gpsimd),
            # then form the f*g map directly in spatial-major on the DVE
            ptg = tp_tile()
            nc.tensor.matmul(ptg[0:1, 0, 0, :], g, ident, is_transpose=True,
                             start=True, stop=True)
            gRow = small.tile([1, 128], dt.bfloat16, tag="gr")
            nc.vector.tensor_copy(gRow, ptg[0:1, 0, 0, :])
            nc.gpsimd.partition_broadcast(gB_all[:, n, :], gRow)
            for b0, b1 in groups:
                gbc = gB_all[:, n:n + 1, :].broadcast_to([128, b1 - b0, 128])
                nc.vector.tensor_mul(fT_all[:, 1, b0:b1, n, :],
                                     fT_all[:, 0, b0:b1, n, :], gbc)

        # ================= phase 2: banded sweeps + back transposes
        ot_slabs = {}

        def back_transpose(b):
            slab = ot_slabs.pop(b)
            pt = tp_tile()
            for m in range(2):
                for j in range(NPC):
                    k = m * NPC + j
                    nc.tensor.matmul(pt[:, m, j, :], slab[:, m, j, :],
                                     ident, is_transpose=True,
                                     start=(k == 0), stop=(k == 7))
            copy_cast(oc[:, :, :, b, :], pt)

        # fuse + bias + store, one (m, j, ot) tuple over a 4-block column
        # span; spread through phase 2 as spans become back-transposed
        _fa = [0]
        TUPLES = [(m, j, ot) for m in range(2) for j in range(NPC)
                  for ot in range(2)]

        def fuse_tuple(tup, s0, width):
            m, j, ot = tup
            ocf = oc[:, m, j].rearrange("p a b -> p (a b)")
            ps = fu_tile().rearrange("p a b -> p (a b)")[:, :width]
            nc.tensor.matmul(ps, wfuseT[:, ot, :], ocf[:, s0:s0 + width],
                             start=True, stop=True)
            yst = ystage.tile([128, 512], dt.bfloat16, tag="yst")
            dst = yst[:, :width]
            _fa[0] ^= 1
            if _fa[0]:
                nc.scalar.activation(dst, ps, AF.Identity,
                                     bias=bfuse[:, ot:ot + 1], scale=1.0)
            else:
                nc.vector.tensor_scalar_add(dst, ps, bfuse[:, ot:ot + 1])
            nc.sync.dma_start(out=y_d[j, m, ot, :, s0:s0 + width], in_=dst)

        def sweep(b):
            ph = b % PH
            slab = otp.tile([128, 2, NPC, 128], dt.bfloat16, tag="ot")
            ot_slabs[b] = slab
            for m in range(2):
                kidA, kidB = (0, 1) if m == 0 else (2, 3)
                ps = acc_tile()
                mms = []
                for pos, d in ((0, -1), (1, 0), (2, 1)):
                    bi = b + d
                    if 0 <= bi < NB:
                        mms.append((kidA, 1, pos, bi))   # A-kernel on f*g
                        mms.append((kidB, 0, pos, bi))   # B-kernel on f
                for i, (kid, mp, pos, bi) in enumerate(mms):
                    nc.tensor.matmul(ps, tslice(kid, ph, pos),
                                     fT_all[:, mp, bi, :, :],
                                     start=(i == 0), stop=(i == len(mms) - 1))
                copy_cast(slab[:, m, :, :], ps)

        # block 24 first so its 16 narrow fuse tuples leave the tail; a
        # work queue spreads each ready span's tuples 4 per iteration
        order = [24] + list(range(24))
        fuse_q = []
        done = set()

        def note_done(b):
            done.add(b)
            if b == 24:
                fuse_q.extend((t, 24 * 128, HW - 24 * 128) for t in TUPLES)
            g = b // 4
            if b % 4 == 3 and all(4 * g + k in done for k in range(4)):
                fuse_q.extend((t, g * 512, 512) for t in TUPLES)

        for i, b in enumerate(order):
            sweep(b)
            if i > 0:
                back_transpose(order[i - 1])
                note_done(order[i - 1])
            for _ in range(min(4, len(fuse_q))):
                fuse_tuple(*fuse_q.pop(0))
        back_transpose(order[-1])
        note_done(order[-1])
        while fuse_q:
            fuse_tuple(*fuse_q.pop(0))

    nc.compile()
    return nc


def _get_module():
    if "nc" not in _CACHE:
        _CACHE["nc"] = _build_module()
    return _CACHE["nc"]


# ---------------------------------------------------------------- entry point
def _run(inputs, trace=False, **kwargs):
    import ml_dtypes
    from concourse.bass_utils import run_bass_kernel_spmd

    nc = _get_module()
    consts = _host_consts(inputs)
    x = np.asarray(inputs["x"], np.float32).reshape(N, C, HW).astype(ml_dtypes.bfloat16)
    in_maps = []
    for i in range(NCORES):
        m = dict(consts)
        m["x"] = np.ascontiguousarray(x[i * NPC:(i + 1) * NPC])
        in_maps.append(m)
    return run_bass_kernel_spmd(nc, in_maps, core_ids=list(range(NCORES)),
                                trace=trace, **kwargs)


def kernel(**inputs):
    res = _run(inputs)
    # y per core: [NPC, 2, 2, 128, HW] -> channel = m*256 + ot*128 + o_local
    y = np.concatenate([np.asarray(r["y"]).reshape(NPC, 2 * P2, HW)
                        for r in res.results], axis=0)
    return y.reshape(N, 2 * P2, H, W).astype(np.float32)


if __name__ == "__main__":
    rng = np.random.default_rng(0)
    demo = {
        "x": rng.standard_normal((N, C, H, W), np.float32),
        "W_conv": 0.05 * rng.standard_normal((CM, C)).astype(np.float32),
        "b_conv": 0.05 * rng.standard_normal(CM).astype(np.float32),
        "wk": 0.05 * rng.standard_normal(25).astype(np.float32),
        "bk": 0.05 * rng.standard_normal(25).astype(np.float32),
        "wck": np.float32(0.03), "bck": np.float32(0.01),
        "wk2": 0.05 * rng.standard_normal(9).astype(np.float32),
        "bk2": 0.05 * rng.standard_normal(9).astype(np.float32),
        "wck2": np.float32(0.02), "bck2": np.float32(-0.01),
        "W_fuse": 0.05 * rng.standard_normal((P2, CM)).astype(np.float32),
        "b_fuse": 0.05 * rng.standard_normal(P2).astype(np.float32),
    }
    out = kernel(**demo)
    print(out.shape, out.dtype)


# revision 27
# speedup vs baseline: 1.0179x; 1.0179x over previous
"""Trainium2 Bass kernel for nn_DIDAModule (dense_cnn).

Math: the per-sample "dynamic" depthwise kernels are affine in the channel
gate g:  kern1 = g*A1 + B1  with  A1 = wk*wck, B1 = bk*wck + bck  (5x5) and
A2 = wk2*wck2, B2 = bk2*wck2 + bck2 (3x3, dilation 2).  Per-channel scaling
commutes with the (channel-shared) depthwise convs, so

    o1 = conv_A1(g*f) + conv_B1(f)      o2 = conv_A2(g*f) + conv_B2(f)
    y  = [W_fuse @ o1 + b_fuse ; W_fuse @ o2 + b_fuse]

The four static depthwise convs are banded spatial matmuls on the Tensor
engine: spatial-major layout, flat 128-pixel blocks, 7 phase classes
(128 mod 56 = 16, period 7), 3 band position matrices (prev/self/next
block) per phase per kernel, built host side from the conv geometry.

Layout changes between channel-major (conv1, fuse) and spatial-major
(banded sweeps) are done with PE-array transposes (is_transpose matmuls
against an identity), packed 8 tiles per PSUM bank — no DMA-xbar
transposes, no DRAM staging, no write-landing delay chains.  Everything
is bf16 with fp32 PSUM accumulation; x is cast to bf16 host-side and y
is returned as bf16 and cast back host-side.

Sharding: data-parallel over batch N across the 8 cores (4 samples each),
weights replicated.
"""

import numpy as np

# ---------------------------------------------------------------- dims
N, C, H, W = 32, 512, 56, 56
CM, K1, K2, P2 = 128, 5, 3, 256
HW = H * W            # 3136
SP = 3200             # padded spatial: 25 blocks of 128
NB = 25
PH = 7                # phase classes
NCORES = 8
NPC = N // NCORES     # samples per core
EW = 448              # conv1 / fuse strip width (7 per map)
NE = 7

_CACHE = {}


# ---------------------------------------------------------------- host prep
def _build_T(K2d, dil):
    """Banded conv matrices T[phase, pos, k_in, m_out] for flat 128-blocks."""
    kh = K2d.shape[0]
    r = (kh - 1) // 2 * dil
    T = np.zeros((PH, 3, 128, 128), np.float32)
    for p in range(PH):
        bref = 7 + p              # interior reference block of this phase
        for pos, d in enumerate((-1, 0, 1)):
            for m in range(128):
                s_out = bref * 128 + m
                ro, wo = divmod(s_out, W)
                for k in range(128):
                    s_in = (bref + d) * 128 + k
                    ri, wi = divmod(s_in, W)
                    di, dj = ri - ro, wi - wo
                    if (abs(di) <= r and abs(dj) <= r
                            and di % dil == 0 and dj % dil == 0):
                        T[p, pos, k, m] = K2d[di // dil + (kh - 1) // 2,
                                              dj // dil + (kh - 1) // 2]
    return T


def _host_consts(inp):
    import ml_dtypes
    bf16 = ml_dtypes.bfloat16
    W_conv = np.asarray(inp["W_conv"], np.float32)     # [CM, C]
    W_fuse = np.asarray(inp["W_fuse"], np.float32)     # [P2, CM]
    A1 = (np.asarray(inp["wk"]) * float(inp["wck"])).reshape(K1, K1)
    B1 = (np.asarray(inp["bk"]) * float(inp["wck"]) + float(inp["bck"])).reshape(K1, K1)
    A2 = (np.asarray(inp["wk2"]) * float(inp["wck2"])).reshape(K2, K2)
    B2 = (np.asarray(inp["bk2"]) * float(inp["wck2"]) + float(inp["bck2"])).reshape(K2, K2)
    # T layout: [k_in(128part), kern(4), ph(7), pos(3), m_out(128)]
    T = np.stack([_build_T(A1.astype(np.float32), 1),
                  _build_T(B1.astype(np.float32), 1),
                  _build_T(A2.astype(np.float32), 2),
                  _build_T(B2.astype(np.float32), 2)])      # [4,7,3,128,128]
    T_h = np.ascontiguousarray(T.transpose(3, 0, 1, 2, 4)).reshape(128, 84 * 128)
    # conv1 lhsT chunks: [c_local(128part), kc(4), cm(128)]
    wconvT_h = np.ascontiguousarray(
        W_conv.T.reshape(4, 128, CM).transpose(1, 0, 2)).reshape(128, 4 * CM)
    # fuse lhsT chunks: [c(128part), chunk(2), o_local(128)]
    wfuseT_h = np.ascontiguousarray(
        W_fuse.T.reshape(CM, 2, 128)).reshape(CM, 256)
    # fuse bias per-partition columns: [o_local(128part), chunk(2)]
    bfuse_h = np.ascontiguousarray(
        np.asarray(inp["b_fuse"], np.float32).reshape(2, 128).T)
    return {
        "wconvT": wconvT_h.astype(bf16),
        "bconv": np.asarray(inp["b_conv"], np.float32).reshape(CM, 1),
        "Tmat": T_h.astype(bf16),
        "wfuseT": wfuseT_h.astype(bf16),
        "bfuse": bfuse_h,
        "ident": np.eye(128, dtype=bf16),
    }


# ---------------------------------------------------------------- bass module
def _build_module():
    from contextlib import ExitStack
    import concourse.bass as bass  # noqa: F401
    import concourse.mybir as mybir
    import concourse.tile as tile
    from concourse import bacc

    dt = mybir.dt
    AX = mybir.AxisListType
    AF = mybir.ActivationFunctionType

    nc = bacc.Bacc("TRN2", target_bir_lowering=False, debug=False)

    x_d = nc.dram_tensor("x", [NPC, C, HW], dt.bfloat16, kind="ExternalInput").ap()
    wconvT_d = nc.dram_tensor("wconvT", [128, 4 * CM], dt.bfloat16, kind="ExternalInput").ap()
    bconv_d = nc.dram_tensor("bconv", [CM, 1], dt.float32, kind="ExternalInput").ap()
    T_d = nc.dram_tensor("Tmat", [128, 84 * 128], dt.bfloat16, kind="ExternalInput").ap()
    wfuseT_d = nc.dram_tensor("wfuseT", [CM, 256], dt.bfloat16, kind="ExternalInput").ap()
    bfuse_d = nc.dram_tensor("bfuse", [128, 2], dt.float32, kind="ExternalInput").ap()
    ident_d = nc.dram_tensor("ident", [128, 128], dt.bfloat16, kind="ExternalInput").ap()
    # y layout: [n, m(2 branches), otile(2), o_local(128), s]
    y_d = nc.dram_tensor("y", [NPC, 2, 2, 128, HW], dt.bfloat16, kind="ExternalOutput").ap()

    with tile.TileContext(nc) as tc, ExitStack() as ctx:
        consts = ctx.enter_context(tc.tile_pool(name="consts", bufs=1))
        xpool = ctx.enter_context(tc.tile_pool(name="xp", bufs=11))
        fpool = ctx.enter_context(tc.tile_pool(name="fp", bufs=1))
        big = ctx.enter_context(tc.tile_pool(name="big", bufs=1))
        otp = ctx.enter_context(tc.tile_pool(name="otp", bufs=2))
        ystage = ctx.enter_context(tc.tile_pool(name="yst", bufs=4))
        small = ctx.enter_context(tc.tile_pool(name="small", bufs=4))
        psA = ctx.enter_context(tc.tile_pool(name="psA", bufs=4, space="PSUM"))
        psB = ctx.enter_context(tc.tile_pool(name="psB", bufs=4, space="PSUM"))

        def acc_tile():
            # uniform [128, 512] fp32 bank; callers slice/reshape views
            return psA.tile([128, 4, 128], dt.float32, tag="acc", name="acc",
                            bufs=3)

        def tp_tile():
            # uniform [128, 2*NPC, 128] bf16 bank for PE transposes
            return psB.tile([128, 2, NPC, 128], dt.bfloat16, tag="tp", name="tp",
                            bufs=3)

        def fu_tile():
            # dedicated fuse ring so fuse copies don't stall the sweep banks
            return psA.tile([128, 4, 128], dt.float32, tag="fu", name="fu",
                            bufs=2)

        # ---- constants to SBUF (small, immediately-needed ones first; the
        # heavy Tm and fuse consts are DMA'd after sample 0's x chunks)
        wconvT = consts.tile([128, 4, CM], dt.bfloat16)
        nc.sync.dma_start(out=wconvT, in_=wconvT_d)
        bconv = consts.tile([CM, 1], dt.float32)
        nc.sync.dma_start(out=bconv, in_=bconv_d)
        ident = consts.tile([128, 128], dt.bfloat16)
        nc.sync.dma_start(out=ident, in_=ident_d)
        # heavy consts ride the Activation HWDGE queue alone; x owns SP
        Tm = consts.tile([128, 84, 128], dt.bfloat16)
        nc.scalar.dma_start(out=Tm, in_=T_d)
        wfuseT = consts.tile([CM, 2, 128], dt.bfloat16)
        nc.scalar.dma_start(out=wfuseT, in_=wfuseT_d)
        bfuse = consts.tile([128, 2], dt.float32)
        nc.scalar.dma_start(out=bfuse, in_=bfuse_d)

        # ---- persistent spatial-major tensors
        # fT_all: [s_loc, map(0=f,1=f*g), b, j, c]
        fT_all = big.tile([128, 2, NB, NPC, 128], dt.bfloat16)
        # oc: [c, m(0=5x5,1=3x3d2), j, b, s_loc]
        oc = big.tile([128, 2, NPC, NB, 128], dt.bfloat16)

        # engine-alternating PSUM->SBUF copy (plain cast copy)
        _alt = [0]

        def copy_cast(dst, src):
            _alt[0] ^= 1
            if _alt[0]:
                nc.scalar.activation(dst, src, AF.Copy)
            else:
                nc.vector.tensor_copy(dst, src)

        def tslice(kid, ph, pos):
            return Tm[:, kid * 21 + ph * 3 + pos, :]

        # gate broadcast rows: gB_all[:, n, :] = g_n[c] replicated over partitions
        gB_all = big.tile([128, NPC, 128], dt.bfloat16)

        # ================= phase 1: conv1 + gate + forward transposes
        def load_x(n):
            xc = []
            for kc in range(4):
                xt = xpool.tile([128, HW], dt.bfloat16, tag="x")
                nc.sync.dma_start(out=xt, in_=x_d[n, kc * 128:(kc + 1) * 128, :])
                xc.append(xt)
            return xc

        xcs = {n: load_x(n) for n in range(3)}   # ~3 samples in the x ring
        for n in range(NPC):
            xc = xcs.pop(n)
            if n == 1:
                xcs[3] = load_x(3)
            # f tile: [c, s_padded]; pad columns must be zero
            f = fpool.tile([128, SP], dt.bfloat16, tag="f")
            nc.gpsimd.memset(f[:, HW:SP], 0.0)
            gpart = small.tile([128, NE], dt.float32, tag="gp")
            # sample 0: strip waves of 3 so the first matmuls only need x
            # chunk 0 (later chunks are still in flight); others strip-major
            waves = [(0, 3), (3, 6), (6, 7)] if n == 0 else [(e, e + 1)
                                                             for e in range(NE)]
            for w0, w1 in waves:
                pss = [acc_tile().rearrange("p a b -> p (a b)")[:, :EW]
                       for _ in range(w0, w1)]
                for kc in range(4):
                    for i, e in enumerate(range(w0, w1)):
                        nc.tensor.matmul(pss[i], wconvT[:, kc, :],
                                         xc[kc][:, e * EW:(e + 1) * EW],
                                         start=(kc == 0), stop=(kc == 3))
                for i, e in enumerate(range(w0, w1)):
                    nc.vector.reduce_sum(gpart[:, e:e + 1], pss[i], axis=AX.X)
                    nc.scalar.activation(f[:, e * EW:(e + 1) * EW], pss[i],
                                         AF.Relu, bias=bconv[:, 0:1], scale=1.0)
            gsum = small.tile([128, 1], dt.float32, tag="gs")
            nc.vector.reduce_sum(gsum, gpart, axis=AX.X)
            g = small.tile([128, 1], dt.bfloat16, tag="g")
            nc.scalar.activation(g, gsum, AF.Relu, bias=bconv[:, 0:1],
                                 scale=1.0 / HW)
            # forward transposes of f
            groups = [(b0, min(b0 + 8, NB)) for b0 in range(0, NB, 8)]
            for b0, b1 in groups:
                pt = tp_tile().rearrange("p a b c -> p (a b) c")
                for k, b in enumerate(range(b0, b1)):
                    nc.tensor.matmul(pt[:, k, :], f[:, b * 128:(b + 1) * 128],
                                     ident, is_transpose=True,
                                     start=(k == 0), stop=(b == b1 - 1))
                copy_cast(fT_all[:, 0, b0:b1, n, :], pt[:, 0:b1 - b0, :])
            # transpose g to a row, broadcast across partitions (gpsimd),
            # then form the f*g map directly in spatial-major on the DVE
            ptg = tp_tile()
            nc.tensor.matmul(ptg[0:1, 0, 0, :], g, ident, is_transpose=True,
                             start=True, stop=True)
            gRow = small.tile([1, 128], dt.bfloat16, tag="gr")
            nc.vector.tensor_copy(gRow, ptg[0:1, 0, 0, :])
            nc.gpsimd.partition_broadcast(gB_all[:, n, :], gRow)
            for b0, b1 in groups:
                gbc = gB_all[:, n:n + 1, :].broadcast_to([128, b1 - b0, 128])
                nc.vector.tensor_mul(fT_all[:, 1, b0:b1, n, :],
                                     fT_all[:, 0, b0:b1, n, :], gbc)

        # ================= phase 2: banded sweeps + back transposes
        ot_slabs = {}

        def back_transpose(b):
            slab = ot_slabs.pop(b)
            pt = tp_tile()
            for m in range(2):
                for j in range(NPC):
                    k = m * NPC + j
                    nc.tensor.matmul(pt[:, m, j, :], slab[:, m, j, :],
                                     ident, is_transpose=True,
                                     start=(k == 0), stop=(k == 7))
            copy_cast(oc[:, :, :, b, :], pt)

        # fuse + bias + store, one (m, j, ot) tuple over a 4-block column
        # span; spread through phase 2 as spans become back-transposed
        _fa = [0]
        TUPLES = [(m, j, ot) for m in range(2) for j in range(NPC)
                  for ot in range(2)]

        def fuse_tuple(tup, s0, width):
            m, j, ot = tup
            ocf = oc[:, m, j].rearrange("p a b -> p (a b)")
            ps = fu_tile().rearrange("p a b -> p (a b)")[:, :width]
            nc.tensor.matmul(ps, wfuseT[:, ot, :], ocf[:, s0:s0 + width],
                             start=True, stop=True)
            yst = ystage.tile([128, 512], dt.bfloat16, tag="yst")
            dst = yst[:, :width]
            _fa[0] ^= 1
            if _fa[0]:
                nc.scalar.activation(dst, ps, AF.Identity,
                                     bias=bfuse[:, ot:ot + 1], scale=1.0)
            else:
                nc.vector.tensor_scalar_add(dst, ps, bfuse[:, ot:ot + 1])
            nc.sync.dma_start(out=y_d[j, m, ot, :, s0:s0 + width], in_=dst)

        def sweep(b):
            ph = b % PH
            slab = otp.tile([128, 2, NPC, 128], dt.bfloat16, tag="ot")
            ot_slabs[b] = slab
            for m in range(2):
                kidA, kidB = (0, 1) if m == 0 else (2, 3)
                ps = acc_tile()
                mms = []
                for pos, d in ((0, -1), (1, 0), (2, 1)):
                    bi = b + d
                    if 0 <= bi < NB:
                        mms.append((kidA, 1, pos, bi))   # A-kernel on f*g
                        mms.append((kidB, 0, pos, bi))   # B-kernel on f
                for i, (kid, mp, pos, bi) in enumerate(mms):
                    nc.tensor.matmul(ps, tslice(kid, ph, pos),
                                     fT_all[:, mp, bi, :, :],
                                     start=(i == 0), stop=(i == len(mms) - 1))
                copy_cast(slab[:, m, :, :], ps)

        # block 24 first so its 16 narrow fuse tuples leave the tail; a
        # work queue spreads each ready span's tuples 4 per iteration
        order = [24] + list(range(24))
        fuse_q = []
        done = set()

        def note_done(b):
            done.add(b)
            if b == 24:
                fuse_q.extend((t, 24 * 128, HW - 24 * 128) for t in TUPLES)
            g = b // 4
            if b % 4 == 3 and all(4 * g + k in done for k in range(4)):
                fuse_q.extend((t, g * 512, 512) for t in TUPLES)

        for i, b in enumerate(order):
            sweep(b)
            if i > 0:
                back_transpose(order[i - 1])
                note_done(order[i - 1])
            for _ in range(min(4, len(fuse_q))):
                fuse_tuple(*fuse_q.pop(0))
        back_transpose(order[-1])
        note_done(order[-1])
        while fuse_q:
            fuse_tuple(*fuse_q.pop(0))

    nc.compile()
    return nc


def _get_module():
    if "nc" not in _CACHE:
        _CACHE["nc"] = _build_module()
    return _CACHE["nc"]


# ---------------------------------------------------------------- entry point
def _run(inputs, trace=False, **kwargs):
    import ml_dtypes
    from concourse.bass_utils import run_bass_kernel_spmd

    nc = _get_module()
    consts = _host_consts(inputs)
    x = np.asarray(inputs["x"], np.float32).reshape(N, C, HW).astype(ml_dtypes.bfloat16)
    in_maps = []
    for i in range(NCORES):
        m = dict(consts)
        m["x"] = np.ascontiguousarray(x[i * NPC:(i + 1) * NPC])
        in_maps.append(m)
    return run_bass_kernel_spmd(nc, in_maps, core_ids=list(range(NCORES)),
                                trace=trace, **kwargs)


def kernel(**inputs):
    res = _run(inputs)
    # y per core: [NPC, 2, 2, 128, HW] -> channel = m*256 + ot*128 + o_local
    y = np.concatenate([np.asarray(r["y"]).reshape(NPC, 2 * P2, HW)
                        for r in res.results], axis=0)
    return y.reshape(N, 2 * P2, H, W).astype(np.float32)


if __name__ == "__main__":
    rng = np.random.default_rng(0)
    demo = {
        "x": rng.standard_normal((N, C, H, W), np.float32),
        "W_conv": 0.05 * rng.standard_normal((CM, C)).astype(np.float32),
        "b_conv": 0.05 * rng.standard_normal(CM).astype(np.float32),
        "wk": 0.05 * rng.standard_normal(25).astype(np.float32),
        "bk": 0.05 * rng.standard_normal(25).astype(np.float32),
        "wck": np.float32(0.03), "bck": np.float32(0.01),
        "wk2": 0.05 * rng.standard_normal(9).astype(np.float32),
        "bk2": 0.05 * rng.standard_normal(9).astype(np.float32),
        "wck2": np.float32(0.02), "bck2": np.float32(-0.01),
        "W_fuse": 0.05 * rng.standard_normal((P2, CM)).astype(np.float32),
        "b_fuse": 0.05 * rng.standard_normal(P2).astype(np.float32),
    }
    out = kernel(**demo)
    print(out.shape, out.dtype)
